# revision 32
# baseline (speedup 1.0000x reference)
"""Trainium2 Bass kernel for nn_BioGNN (3-layer GAT + mean-pool + linear head).

8-core SPMD strategy:
  - Nodes sharded into 8 contiguous ranges (6250/core, padded to 6272=49*128).
  - Per layer: dense transform on PE (augmented weights also produce per-node
    attention terms e_src/e_dst as extra columns); AllGather the per-node bf16
    rows [h(256)|e_src(4)|e_dst(4)|pad->384]; edge phase gathers src rows by
    indirect DMA (768B/row), computes ex=exp(leakyrelu(e_src+e_dst)) per edge,
    and scatter-accumulates [Sum(ex*h_src) | Sum(ex)] per dst node via one
    one-hot matmul per tile on PE; epilogue normalizes, adds bias, applies ELU
    (layers 1-2).
  - Pooling via one-hot(batch) matmul + AllReduce + linear head (fp32).

Most edge-phase tensors are bf16 (tolerance is 2e-2; bf16 keeps ~3e-3).
Per-block DVE ops are batched via free-dim-broadcast access patterns.
Softmax skips the segment-max shift (logits are O(10), exp is safe in bf16's
range and alpha is shift-invariant).
"""
import os
import sys

for _p in ("/opt/trn_rl_repo", "/root/.axon_site/_ro/trn_rl_repo"):
    if _p not in sys.path:
        sys.path.insert(0, _p)

import numpy as np
import ml_dtypes

import concourse.bass as bass
import concourse.tile as tile
from concourse import bacc, mybir
from concourse.bass_utils import run_bass_kernel_spmd
from concourse.library_config import mlp as mlp_lib

P = 128
NCORES = 8
F32 = mybir.dt.float32
BF = mybir.dt.bfloat16
U8 = mybir.dt.uint8
F8 = mybir.dt.float8e4
I16 = mybir.dt.int16
I32 = mybir.dt.int32
AF = mybir.ActivationFunctionType
ALU = mybir.AluOpType
BF_NP = ml_dtypes.bfloat16
F8_NP = ml_dtypes.float8_e4m3

# problem config (hardcoded per spec); tests may build scaled-down variants
CFG = dict(N=50000, G=64, IN=128, HID=64, H=4, OUT=10)
ABLATE = set(os.environ.get("KABLATE", "").split(",")) - {""}
ROW = 264                      # h(256) | e_src(4) | e_dst(4)
ROWP = 384                     # bf16 row padded to a 256B multiple (768B)
GCHUNK = 8                     # gather-call size in tiles (128 idxs each)
SCRATCH = 0                    # SWDGE scratch override (0 = default 16KB)
NQUEUES = 1                    # SWDGE queues for gather DMA
NCHUNK = 4                     # AllGather chunks per layer (1 = bulk)


def build_program(TBS, cfg=CFG, sim_single=False):
    """Build the SPMD program. TBS: per-block tile counts (len NB), identical
    across cores. sim_single=True builds a 1-device timing model (collectives
    replaced by local copies) for TimelineSim analysis only."""
    N, G, IN, HID, H, OUTF = (cfg["N"], cfg["G"], cfg["IN"], cfg["HID"],
                              cfg["H"], cfg["OUT"])
    F = H * HID
    NSH = N // NCORES
    NB = (NSH + P - 1) // P
    NSHP = NB * P
    assert len(TBS) == NB and all(len(t) == 2 for t in TBS)
    TBSUM = [lo + hi for lo, hi in TBS]
    TT = sum(TBSUM)
    TBMAX = max(TBSUM)
    NCALLS = sum((tg + GCHUNK - 1) // GCHUNK for pair in TBS for tg in pair)
    KT = F // P                    # K-tiles for layers 2-3 (2)

    kw = dict(dynamic_dma_scratch_size=SCRATCH) if SCRATCH else {}
    if NQUEUES > 1:
        kw["num_swdge_queues"] = NQUEUES
    nc = bacc.Bacc("TRN2", target_bir_lowering=False, debug=False,
                   num_devices=1 if sim_single else NCORES, **kw)

    # ---- I/O ----
    xT = nc.dram_tensor("xT", [IN, NSHP], BF, kind="ExternalInput")
    idx16 = nc.dram_tensor("idx16", [P, 8 * TT], I16, kind="ExternalInput")
    ptall_in = nc.dram_tensor("ptall8", [P, TT * P], F8, kind="ExternalInput")
    dstloc = nc.dram_tensor("dstloc", [P, TT], F32, kind="ExternalInput")
    gcnt = nc.dram_tensor("gcnt", [1, NCALLS], I32, kind="ExternalInput")
    iotap_in = nc.dram_tensor("iotap", [P, 1], F32, kind="ExternalInput")
    batchloc = nc.dram_tensor("batchloc", [P, NB], F32, kind="ExternalInput")
    iota_in = nc.dram_tensor("iota", [P, P], BF, kind="ExternalInput")
    identb_in = nc.dram_tensor("identb", [P, P], BF, kind="ExternalInput")
    identf_in = nc.dram_tensor("identf", [G, G], F32, kind="ExternalInput")
    wts = [nc.dram_tensor(f"wt{l}", [IN if l == 1 else F, ROW], BF,
                          kind="ExternalInput") for l in (1, 2, 3)]
    breps = [nc.dram_tensor(f"brep{l}", [P, F], BF, kind="ExternalInput")
             for l in (1, 2, 3)]
    wlt = nc.dram_tensor("wlt", [F, OUTF], F32, kind="ExternalInput")
    blrep = nc.dram_tensor("blrep", [G, OUTF], F32, kind="ExternalInput")
    invcnt = nc.dram_tensor("invcnt", [G, 1], F32, kind="ExternalInput")
    out_ext = nc.dram_tensor("out", [G, OUTF], F32, kind="ExternalOutput")

    # ---- internal DRAM ----
    shr = {} if sim_single else dict(addr_space="Shared")
    hrow_own = [nc.dram_tensor(f"hrow_own{i}", [NSHP, ROWP], BF)
                for i in (0, 1, 2)]
    hrow_full = [nc.dram_tensor(f"hrow_full{i}", [NCORES * NSHP, ROWP], BF,
                                **shr) for i in (0, 1, 2)]
    pool_own = nc.dram_tensor("pool_own", [G, F], F32)
    pool_full = nc.dram_tensor("pool_full", [G, F], F32, **shr)

    with tile.TileContext(nc) as tc:
        with (
            tc.tile_pool(name="const", bufs=1) as cpool,
            tc.tile_pool(name="wpool", bufs=2) as wpool,
            tc.tile_pool(name="sb", bufs=3) as pool,
            tc.tile_pool(name="gpool", bufs=2) as gpool,
            tc.tile_pool(name="gbuf", bufs=3) as gbuf,
            tc.tile_pool(name="ps", bufs=2, space="PSUM") as pspool,
            tc.tile_pool(name="ps1", bufs=1, space="PSUM") as psone,
            tc.tile_pool(name="pspool1", bufs=1, space="PSUM") as pspool1,
        ):
            # resident constants
            iota_sb = cpool.tile([P, P], BF)
            nc.sync.dma_start(iota_sb[:], iota_in[:])
            identb_sb = cpool.tile([P, P], BF)
            nc.sync.dma_start(identb_sb[:], identb_in[:])
            identf_sb = cpool.tile([G, G], F32)
            nc.sync.dma_start(identf_sb[:], identf_in[:])
            idx16_sb = cpool.tile([P, 8 * TT], I16)
            nc.sync.dma_start(idx16_sb[:], idx16[:])
            iotap_sb = cpool.tile([P, 1], F32)
            nc.sync.dma_start(iotap_sb[:], iotap_in[:])
            dstloc_sb = cpool.tile([P, TT], F32)
            nc.sync.dma_start(dstloc_sb[:], dstloc[:])
            batchloc_sb = cpool.tile([P, NB], F32)
            nc.sync.dma_start(batchloc_sb[:], batchloc[:])
            gcnt_sb = cpool.tile([1, NCALLS], I32)
            nc.sync.dma_start(gcnt_sb[:], gcnt[:])
            nc.gpsimd.load_library(mlp_lib)
            tc.strict_bb_all_engine_barrier()

            for _ in range(3):
                g0 = gbuf.tile([P, TBMAX * ROWP], BF, tag="gath")
                nc.vector.memset(g0[:], 0.0)
            creg = nc.gpsimd.alloc_register("gcnt_reg")

            pool_ps = pspool1.tile([G, F], mybir.dt.float32, tag="pool")

            # software pipeline: layer l's edge phase (C) carries layer
            # l+1's dense phase (A) and chunked AllGathers, hiding the
            # collective behind edge compute.  hrow tensors ping-pong by
            # layer parity; the gather table uses a chunked layout
            # (chunk-major, then core) so each AllGather chunk lands in a
            # contiguous slice of the table.
            CHB = [NB * q // NCHUNK for q in range(NCHUNK + 1)]
            if NCHUNK == 1:
                HB = NCORES * NSHP // 2       # core-major: mid-table cut
            else:
                HB = NCORES * CHB[NCHUNK // 2] * P
            NTAB = NCORES * NSHP

            def load_wts(l):
                kt_ = 1 if l == 1 else KT
                lst = []
                for k in range(kt_):
                    w = wpool.tile([P, ROW], BF, tag=f"wt{k}")
                    nc.sync.dma_start(w[:], wts[l - 1][k * P:(k + 1) * P, :])
                    lst.append(w)
                br = wpool.tile([P, F], BF, tag="brep")
                nc.sync.dma_start(br[:], breps[l - 1][:])
                return lst, br

            def phase_a(l, b, wt_l, lhsT_tiles=None):
                par = l - 1
                kt_ = 1 if l == 1 else KT
                hlin_ps = psone.tile([P, ROW], mybir.dt.float32, tag="mmA")
                for k in range(kt_):
                    if lhsT_tiles is not None:
                        lt = lhsT_tiles[k]
                    else:
                        lt = pool.tile([P, P], BF, tag="lhsT")
                        nc.scalar.dma_start(lt[:], xT[:, b * P:(b + 1) * P])
                    nc.tensor.matmul(hlin_ps[:], lhsT=lt[:], rhs=wt_l[k][:],
                                     start=(k == 0), stop=(k == kt_ - 1))
                hrow_sb = pool.tile([P, ROW], BF, tag="hrow")
                nc.scalar.activation(hrow_sb[:], hlin_ps[:], AF.Copy)
                nc.sync.dma_start(hrow_own[par][b * P:(b + 1) * P, :ROW],
                                  hrow_sb[:])

            def ag_chunk(l, q):
                par = l - 1
                r0, r1 = CHB[q] * P, CHB[q + 1] * P
                if sim_single:
                    nc.sync.dma_start(
                        hrow_full[par][NCORES * r0:NCORES * r0 + (r1 - r0), :],
                        hrow_own[par][r0:r1, :])
                elif "ag" in ABLATE:
                    pass
                else:
                    nc.gpsimd.collective_compute(
                        "AllGather", ALU.bypass,
                        ins=[hrow_own[par][r0:r1, :]],
                        outs=[hrow_full[par][NCORES * r0:NCORES * r1, :]],
                        replica_groups=[list(range(NCORES))],
                    )

            # prologue: layer 1 dense + its (exposed) chunked AllGather
            wt_cur, brep_cur = load_wts(1)
            q = 0
            for b in range(NB):
                phase_a(1, b, wt_cur)
                if b == CHB[q + 1] - 1:
                    ag_chunk(1, q)
                    q += 1

            for layer in (1, 2, 3):
                par = layer - 1
                if layer < 3:
                    wt_nxt, brep_nxt = load_wts(layer + 1)

                # ---- edge phase (+ piggybacked next-layer dense) ----
                t0 = 0
                call_i = 0
                q = 0
                for b in range(NB):
                    Tb = TBSUM[b]
                    # gather: chunked calls per half; per-core actual edge
                    # counts via register (trailing -1 idxs are skipped)
                    gath = gbuf.tile([P, TBMAX * ROWP], BF, tag="gath")
                    goff = 0
                    for half in (0, 1):
                        Tg = TBS[b][half]
                        if Tg == 0:
                            continue
                        if "gather" in ABLATE:
                            goff += Tg
                            call_i += (Tg + GCHUNK - 1) // GCHUNK
                            continue
                        # dma_gather descriptor-ring limit: chunk calls
                        done = 0
                        while done < Tg:
                            ck = min(GCHUNK, Tg - done)
                            o = goff + done
                            if "notrunc" not in ABLATE:
                                nc.gpsimd.reg_load(
                                    creg, gcnt_sb[0:1, call_i:call_i + 1])
                            nc.gpsimd.dma_gather(
                                out_ap=gath[:, o * ROWP:(o + ck) * ROWP]
                                    .rearrange("p (t e) -> p t e", e=ROWP),
                                in_ap=(hrow_full[par][0:HB, :] if half == 0
                                       else hrow_full[par][HB:NTAB, :]),
                                idxs_ap=idx16_sb[:, 8 * (t0 + o):8 * (t0 + o + ck)],
                                num_idxs=ck * P,
                                num_idxs_reg=(ck * P if "notrunc" in ABLATE
                                              else creg),
                                elem_size=ROWP,
                                queue_num=call_i % NQUEUES,
                            )
                            done += ck
                            call_i += 1
                        goff += Tg

                    # one-hot operands (independent of gathered data):
                    # ptall (dst-on-partition) streamed as fp8 from host;
                    # pmat (dst-on-free) built per tile at 4x on DVE
                    ptall = gpool.tile([P, Tb * P], F8, tag="ptall")
                    if "ptload" not in ABLATE:
                        nc.scalar.dma_start(
                            ptall[:], ptall_in[:, t0 * P:(t0 + Tb) * P])
                    pmat = gpool.tile([P, Tb * P], BF, tag="pmat")
                    if "pbuild" not in ABLATE:
                        for t in range(Tb):
                            nc.vector.tensor_scalar(
                                out=pmat[:, t * P:(t + 1) * P], in0=iota_sb[:],
                                scalar1=dstloc_sb[:, t0 + t:t0 + t + 1],
                                scalar2=None, op0=ALU.is_equal)

                    # e_dst expansion via one-hot matmuls
                    edb = pool.tile([P, 4], BF, tag="edb")
                    nc.scalar.dma_start(
                        edb[:], hrow_own[par][b * P:(b + 1) * P, F + 4:F + 8])
                    edst_ps = pspool.tile([P, 4 * Tb], mybir.dt.float32,
                                          tag="edst")
                    if "edstmm" not in ABLATE:
                        for t in range(Tb):
                            nc.tensor.matmul(edst_ps[:, 4 * t:4 * t + 4],
                                             lhsT=ptall[:, t * P:(t + 1) * P],
                                             rhs=edb[:], start=True, stop=True)

                    # ex = exp(leakyrelu(e_src + e_dst))
                    lg = pool.tile([P, 4 * Tb], BF, tag="lg")
                    nc.vector.tensor_tensor(
                        out=lg[:].rearrange("p (t f) -> p t f", f=4),
                        in0=gath[:, :Tb * ROWP]
                            .rearrange("p (t e) -> p t e", e=ROWP)
                            [:, :, F:F + 4],
                        in1=edst_ps[:].rearrange("p (t f) -> p t f", f=4),
                        op=ALU.add)
                    lr = pool.tile([P, 4 * Tb], BF, tag="lr")
                    nc.vector.scalar_tensor_tensor(
                        out=lr[:], in0=lg[:], scalar=0.2, in1=lg[:],
                        op0=ALU.mult, op1=ALU.max)
                    ex = pool.tile([P, 4 * Tb], BF, tag="ex")
                    if "exp" not in ABLATE:
                        nc.scalar.activation(ex[:], lr[:], AF.Exp)

                    # msg = [h*ex | ex] per tile
                    msg = gpool.tile([P, Tb * (F + 4)], BF, tag="msg")
                    if "muls" not in ABLATE:
                        # c-major feature layout keeps every operand's last AP
                        # dim packed (stride 1) -> DVE 2x mode
                        nc.vector.tensor_tensor(
                            out=msg[:].rearrange("p (t e) -> p t e", e=F + 4)
                                [:, :, 0:F].rearrange("p t (c h) -> p t c h",
                                                      h=H),
                            in0=gath[:, :Tb * ROWP]
                                .rearrange("p (t e) -> p t e", e=ROWP)
                                [:, :, 0:F].rearrange("p t (c h) -> p t c h",
                                                      h=H),
                            in1=ex[:].rearrange("p (t h) -> p t h", h=H)
                                .unsqueeze(2).broadcast_to([P, Tb, HID, H]),
                            op=ALU.mult)
                        nc.vector.tensor_copy(
                            msg[:].rearrange("p (t e) -> p t e", e=F + 4)
                                [:, :, F:F + 4],
                            ex[:].rearrange("p (t h) -> p t h", h=H))

                    # scatter-accumulate [numer | denom] per dst
                    numer_ps = pspool.tile([P, F + 4], mybir.dt.float32,
                                           tag="mm")
                    if "scatter" not in ABLATE:
                        for t in range(Tb):
                            nc.tensor.matmul(
                                numer_ps[:],
                                lhsT=pmat[:, t * P:(t + 1) * P],
                                rhs=msg[:, t * (F + 4):(t + 1) * (F + 4)],
                                start=(t == 0), stop=(t == Tb - 1))

                    # epilogue: y = numer/denom + b
                    dsum = pool.tile([P, H], F32, tag="dsum")
                    # guard pad nodes (zero in-degree): denom=0 -> inf -> NaN
                    nc.vector.tensor_scalar_max(dsum[:], numer_ps[:, F:F + 4],
                                                1e-12)
                    rec = pool.tile([P, H], F32, tag="rec")
                    nc.vector.reciprocal(rec[:], dsum[:])
                    recb = pool.tile([P, H], BF, tag="recb")
                    nc.vector.tensor_copy(recb[:], rec[:])
                    nbf = pool.tile([P, F], BF, tag="nbf")
                    nc.scalar.activation(nbf[:], numer_ps[:, 0:F], AF.Copy)
                    y = pool.tile([P, F], BF, tag="y")
                    nc.vector.tensor_tensor(
                        out=y[:].rearrange("p (c h) -> p c h", h=H),
                        in0=nbf[:].rearrange("p (c h) -> p c h", h=H),
                        in1=recb[:].unsqueeze(1).broadcast_to([P, HID, H]),
                        op=ALU.mult)
                    nc.vector.tensor_tensor(out=y[:], in0=y[:], in1=brep_cur[:],
                                            op=ALU.add)
                    if layer < 3:
                        # ELU: relu(y) + exp(min(y,0)) - 1
                        mn = pool.tile([P, F], BF, tag="mn")
                        nc.vector.tensor_scalar_min(mn[:], y[:], 0.0)
                        eu = pool.tile([P, F], BF, tag="eu")
                        nc.scalar.activation(eu[:], mn[:], AF.Exp)
                        rl = pool.tile([P, F], BF, tag="rl")
                        nc.scalar.activation(rl[:], y[:], AF.Relu)
                        hv = pool.tile([P, F], BF, tag="hv")
                        nc.vector.scalar_tensor_tensor(
                            out=hv[:], in0=eu[:], scalar=-1.0, in1=rl[:],
                            op0=ALU.add, op1=ALU.add)
                        # transpose; the tps tiles feed the next layer's
                        # dense phase directly from SBUF
                        tps_tiles = []
                        for k in range(KT):
                            tp = psone.tile([P, P], BF, tag="tp")
                            nc.tensor.transpose(tp[:], hv[:, k * P:(k + 1) * P],
                                                identb_sb[:])
                            tps = pool.tile([P, P], BF, tag="tps")
                            nc.scalar.activation(tps[:], tp[:], AF.Copy)
                            tps_tiles.append(tps)
                        # next layer's dense phase + chunked AllGather
                        phase_a(layer + 1, b, wt_nxt, lhsT_tiles=tps_tiles)
                        if b == CHB[q + 1] - 1:
                            ag_chunk(layer + 1, q)
                            q += 1
                    else:
                        # pooling accumulation
                        bmat = pool.tile([P, G], BF, tag="bmat")
                        nc.vector.tensor_scalar(
                            out=bmat[:], in0=iota_sb[:, :G],
                            scalar1=batchloc_sb[:, b:b + 1],
                            scalar2=None, op0=ALU.is_equal)
                        nc.tensor.matmul(pool_ps[:], lhsT=bmat[:], rhs=y[:],
                                         start=(b == 0), stop=(b == NB - 1))
                    t0 += Tb
                if layer < 3:
                    wt_cur, brep_cur = wt_nxt, brep_nxt

            # ---- final: pool -> AllReduce -> mean -> linear ----
            pool_sb = pool.tile([G, F], F32, tag="poolsb")
            nc.vector.tensor_copy(pool_sb[:], pool_ps[:])
            nc.sync.dma_start(pool_own[:], pool_sb[:])
            if sim_single:
                nc.sync.dma_start(pool_full[:], pool_own[:])
            else:
                nc.gpsimd.collective_compute(
                    "AllReduce", ALU.add,
                    ins=[pool_own[:]], outs=[pool_full[:]],
                    replica_groups=[list(range(NCORES))],
                )
            invcnt_sb = cpool.tile([G, 1], F32)
            nc.sync.dma_start(invcnt_sb[:], invcnt[:])
            wlt_sb = []
            for k in range(KT):
                w = cpool.tile([P, OUTF], F32)
                nc.sync.dma_start(w[:], wlt[k * P:(k + 1) * P, :])
                wlt_sb.append(w)
            blrep_sb = cpool.tile([G, OUTF], F32)
            nc.sync.dma_start(blrep_sb[:], blrep[:])

            pooled = pool.tile([G, F], F32, tag="pooled")
            nc.sync.dma_start(pooled[:], pool_full[:])
            mean = pool.tile([G, F], F32, tag="mean")
            nc.vector.tensor_scalar_mul(mean[:], pooled[:], invcnt_sb[:])
            fin_ps = psone.tile([G, OUTF], mybir.dt.float32, tag="fin")
            for k in range(KT):
                ptp = psone.tile([P, G], mybir.dt.float32, tag="mmA")
                nc.tensor.transpose(ptp[:], mean[:, k * P:(k + 1) * P],
                                    identf_sb[:])
                ptps = pool.tile([P, G], F32, tag="ptps")
                nc.vector.tensor_copy(ptps[:], ptp[:])
                nc.tensor.matmul(fin_ps[:], lhsT=ptps[:], rhs=wlt_sb[k][:],
                                 start=(k == 0), stop=(k == KT - 1))
            outv = pool.tile([G, OUTF], F32, tag="outv")
            nc.vector.tensor_tensor(out=outv[:], in0=fin_ps[:], in1=blrep_sb[:],
                                    op=ALU.add)
            nc.sync.dma_start(out_ext[:], outv[:])

    nc.compile()
    return nc


def preprocess(x, edge_index, batch, params, cfg=CFG):
    """Host-side index preprocessing + param packing -> (TBS, in_maps)."""
    N, G, IN, HID, H, OUTF = (cfg["N"], cfg["G"], cfg["IN"], cfg["HID"],
                              cfg["H"], cfg["OUT"])
    F = H * HID
    NSH = N // NCORES
    NB = (NSH + P - 1) // P
    NSHP = NB * P

    # chunked gather-table layout: chunk-major, then core, then row
    offs = np.array([NB * qq // NCHUNK * P for qq in range(NCHUNK + 1)],
                    np.int64)
    HALFB = (NCORES * NSHP // 2 if NCHUNK == 1
             else NCORES * int(offs[NCHUNK // 2]))
    src = np.concatenate([np.asarray(edge_index[0]), np.arange(N)]).astype(np.int64)
    dst = np.concatenate([np.asarray(edge_index[1]), np.arange(N)]).astype(np.int64)
    batch = np.asarray(batch).astype(np.int64)

    def remap(nodes):
        c = nodes // NSH
        r = nodes % NSH
        q = np.searchsorted(offs, r, side="right") - 1
        return (NCORES * offs[q] + c * (offs[q + 1] - offs[q])
                + (r - offs[q]))

    core_of = dst // NSH
    tiles_lo = np.zeros((NCORES, NB), np.int64)
    tiles_hi = np.zeros((NCORES, NB), np.int64)
    per_core = []
    for c in range(NCORES):
        m = core_of == c
        s_c, d_c = remap(src[m]), dst[m] - c * NSH
        # sort by (block, half, dst) so each block is lo-group then hi-group
        half_c = (s_c >= HALFB).astype(np.int64)
        blk = d_c // P
        order = np.lexsort((d_c, half_c, blk))
        s_c, d_c, half_c, blk = s_c[order], d_c[order], half_c[order], blk[order]
        cnt_lo = np.bincount(blk[half_c == 0], minlength=NB)
        cnt_hi = np.bincount(blk[half_c == 1], minlength=NB)
        tiles_lo[c] = (cnt_lo + P - 1) // P
        tiles_hi[c] = (cnt_hi + P - 1) // P
        per_core.append((s_c, d_c, half_c, blk, cnt_lo, cnt_hi))
    TBS = [(int(max(tiles_lo[:, b].max(), 1)), int(tiles_hi[:, b].max()))
           for b in range(NB)]
    TBSUM = [lo + hi for lo, hi in TBS]
    TT = sum(TBSUM)
    tb0 = np.cumsum([0] + TBSUM[:-1])
    tbhi0 = [tb0[b] + TBS[b][0] for b in range(NB)]  # first hi tile per block

    W = {k: np.asarray(v, np.float64) for k, v in params.items()}
    # c-major feature permutation: new position i holds original feature
    # (i%H)*HID + i//H.  Keeps DVE last-dims packed in the edge phase.
    CM = np.array([(i % H) * HID + i // H for i in range(F)])
    wt_aug = {}
    for l in (1, 2, 3):
        Wl = W[f"W{l}"]
        asrc, adst = W[f"a_src{l}"], W[f"a_dst{l}"]
        Ablk_s = np.zeros((F, H))
        Ablk_d = np.zeros((F, H))
        for h in range(H):
            Ablk_s[h * HID:(h + 1) * HID, h] = asrc[h]
            Ablk_d[h * HID:(h + 1) * HID, h] = adst[h]
        wa = np.concatenate([Wl.T[:, CM], Wl.T @ Ablk_s, Wl.T @ Ablk_d], axis=1)
        if l > 1:
            wa = wa[CM, :]        # input rows follow prev layer's layout
        wt_aug[l] = wa.astype(BF_NP)

    counts = np.bincount(batch, minlength=G).astype(np.float64)
    invcnt = (1.0 / np.maximum(counts, 1.0)).astype(np.float32)[:, None]
    iota = np.tile(np.arange(P, dtype=np.float32), (P, 1))

    in_maps = []
    xarr = np.asarray(x)
    for c in range(NCORES):
        s_c, d_c, half_c, blk, cnt_lo, cnt_hi = per_core[c]
        # slot index within the (block, half) group
        grp_key = blk * 2 + half_c
        grp_cnt = np.bincount(grp_key, minlength=2 * NB)
        grp_start = np.concatenate([[0], np.cumsum(grp_cnt)[:-1]])
        pos_in_grp = np.arange(len(d_c)) - grp_start[grp_key]
        grp_t0 = np.where(half_c == 0, tb0[blk], np.asarray(tbhi0)[blk])
        t_idx = (grp_t0 + pos_in_grp // P).astype(np.int64)
        p_idx = (pos_in_grp % P).astype(np.int64)

        dstloc = np.full((P, TT), -1.0, np.float32)
        dstloc[p_idx, t_idx] = (d_c - blk * P).astype(np.float32)
        dstrow = np.ascontiguousarray(dstloc.T).reshape(TT * P)
        ptall8 = np.zeros((P, TT * P), F8_NP)
        cols = np.nonzero(dstrow >= 0)[0]
        ptall8[dstrow[cols].astype(np.int64), cols] = F8_NP(1.0)

        # int16 wrapped gather indices: slot j of tile t -> column 8*t + j//16,
        # partitions p with p%16 == j%16 (replicated across the 8 groups).
        # Pads are -1 (skipped by the DGE); per-call valid counts in gcnt.
        trunc = "notrunc" not in ABLATE
        idxflat = np.full(TT * P, -1 if trunc else 0, np.int16)
        idxflat[t_idx * P + p_idx] = (s_c - half_c * HALFB).astype(np.int16)
        gcnt = []
        for b in range(NB):
            for half in (0, 1):
                Tg = TBS[b][half]
                if Tg == 0:
                    continue
                cnt_grp = int((cnt_lo if half == 0 else cnt_hi)[b])
                gs = (tb0[b] if half == 0 else tbhi0[b]) * P
                done = 0
                while done < Tg:
                    ck = min(GCHUNK, Tg - done)
                    if trunc:
                        c_call = max(0, min(cnt_grp - done * P, ck * P))
                        if c_call == 0:
                            idxflat[gs + done * P] = 0  # keep >=1 valid idx
                            c_call = 1
                    else:
                        c_call = ck * P
                    gcnt.append(c_call)
                    done += ck
        gcnt = np.asarray(gcnt, np.int32)[None, :]
        idx16 = np.ascontiguousarray(
            np.tile(idxflat.reshape(TT * 8, 16).T, (8, 1))).astype(np.int16)

        batchloc = np.full(NSHP, -1.0, np.float32)
        batchloc[:NSH] = batch[c * NSH:(c + 1) * NSH]
        batchloc = np.ascontiguousarray(batchloc.reshape(NB, P).T)

        xT_own = np.zeros((IN, NSHP), BF_NP)
        xT_own[:, :NSH] = xarr[c * NSH:(c + 1) * NSH].astype(BF_NP).T

        in_maps.append(dict(
            xT=xT_own, idx16=idx16, ptall8=ptall8,
            dstloc=dstloc, gcnt=gcnt,
            iotap=np.arange(P, dtype=np.float32)[:, None],
            batchloc=batchloc, iota=iota.astype(BF_NP),
            identb=np.eye(P, dtype=BF_NP),
            identf=np.eye(G, dtype=np.float32),
            wt1=wt_aug[1], wt2=wt_aug[2], wt3=wt_aug[3],
            brep1=np.tile(W["b1"][CM].astype(BF_NP), (P, 1)),
            brep2=np.tile(W["b2"][CM].astype(BF_NP), (P, 1)),
            brep3=np.tile(W["b3"][CM].astype(BF_NP), (P, 1)),
            wlt=np.ascontiguousarray(W["Wl"].T[CM, :].astype(np.float32)),
            blrep=np.tile(W["bl"].astype(np.float32), (G, 1)),
            invcnt=invcnt,
        ))
    return TBS, in_maps


def kernel(**inputs):
    x = inputs.pop("x")
    edge_index = inputs.pop("edge_index")
    batch = inputs.pop("batch")
    TBS, in_maps = preprocess(x, edge_index, batch, inputs)
    nc = build_program(TBS)
    res = run_bass_kernel_spmd(nc, in_maps, list(range(NCORES)))
    return np.asarray(res.results[0]["out"], np.float32)
